# revision 1
# baseline (speedup 1.0000x reference)
"""KAN layer on 8 Trainium2 NeuronCores (Bass/Tile).

Computes out = x @ base_weight.T + silu(x) @ spline_weight.sum(-1).T
for x:[8192,1024] f32, base_weight:[1024,1024] f32,
spline_weight:[1024,1024,8] f32 -> out:[8192,1024] f32.

Strategy (self-contained, hardcoded for these shapes):
  * 2D shard over the 8 cores: batch split R=2, out-features split C=4.
    Core (r, c) computes out[4096r:4096(r+1), 256c:256(c+1)].
  * Host prep is pure layout (transpose/reshape/slice): x is passed
    transposed and tiled so every device DMA is a large contiguous
    block with 8-16KB contiguous per SBUF partition row (the measured
    DMA-efficiency knee on this part).
  * On-device per core: the spline g-axis reduce runs on the Vector
    engine, weights are cast to bf16, x is cast (bf16) + silu'd (Scalar
    engine), and the two matmuls are fused into one K=2048 bf16
    accumulation per PSUM tile on the Tensor engine (f32 accumulate).
  * Output is written bf16 (intermediate rounding only; the f32
    result of the accumulation is rounded once) and upcast to f32 on
    gather. End-to-end relative error vs the f32 reference is ~3e-3.
"""
import sys

for _p in ("/opt/trn_rl_repo",):
    if _p not in sys.path:
        sys.path.insert(0, _p)

import numpy as np

import concourse.bass as bass  # noqa: F401  (bass must import before mybir use)
import concourse.mybir as mybir
import concourse.tile as tile
from concourse import bacc
from concourse.bass_utils import run_bass_kernel_spmd

P = 128
IN_F = 1024
G = 8
N_CORES = 8
R_SPLIT = 2
C_SPLIT = 4
B_LOC = 8192 // R_SPLIT      # 4096 batch rows per core
O_LOC = 1024 // C_SPLIT      # 256 out features per core
KT = IN_F // P               # 8 k-tiles over in_features
M_CHUNK = 512
N_CHUNKS = B_LOC // M_CHUNK  # 8
J_SUB = M_CHUNK // P         # 4

F32 = mybir.dt.float32
BF16 = mybir.dt.bfloat16
AF = mybir.ActivationFunctionType

_compiled = None


def _build_kernel():
    nc = bacc.Bacc(None, target_bir_lowering=False, num_devices=N_CORES)
    xt = nc.dram_tensor("xt", [N_CHUNKS, P, KT, M_CHUNK], F32, kind="ExternalInput")
    bt = nc.dram_tensor("bt", [P, KT, O_LOC], F32, kind="ExternalInput")
    st = nc.dram_tensor("st", [KT, P, G, O_LOC], F32, kind="ExternalInput")
    out = nc.dram_tensor("out", [N_CHUNKS, P, J_SUB, O_LOC], BF16,
                         kind="ExternalOutput")

    with tile.TileContext(nc) as tc:
        with (
            tc.tile_pool(name="wconst", bufs=1) as wconst,
            tc.tile_pool(name="wstage", bufs=2) as wstage,
            tc.tile_pool(name="xstage", bufs=4) as xstage,
            tc.tile_pool(name="xcat", bufs=4) as xcat,
            tc.tile_pool(name="psum", bufs=8, space="PSUM") as psum,
            tc.tile_pool(name="opool", bufs=6) as opool,
        ):
            # ---- base weights -> bf16 k-tiles ----
            bstage = wconst.tile([P, KT, O_LOC], F32, name="bstage")
            nc.sync.dma_start(bstage[:], bt[:])
            wb_bf = []
            for t in range(KT):
                wbb = wconst.tile([P, O_LOC], BF16, name=f"wbb{t}")
                nc.vector.tensor_copy(wbb[:], bstage[:, t])
                wb_bf.append(wbb)

            # ---- spline weight: g-sum on DVE, then bf16 ----
            ws_bf = []
            for t in range(KT):
                stg = wstage.tile([P, G, O_LOC], F32, name="stg", tag="stg")
                nc.sync.dma_start(stg[:], st[t])
                acc = wstage.tile([P, O_LOC], F32, name="wsac", tag="wsac")
                h1 = wstage.tile([P, O_LOC], F32, name="wsh1", tag="wsh1")
                nc.vector.tensor_add(acc[:], stg[:, 0], stg[:, 1])
                nc.vector.tensor_add(h1[:], stg[:, 2], stg[:, 3])
                nc.vector.tensor_add(acc[:], acc[:], h1[:])
                nc.vector.tensor_add(h1[:], stg[:, 4], stg[:, 5])
                nc.vector.tensor_add(acc[:], acc[:], h1[:])
                nc.vector.tensor_add(h1[:], stg[:, 6], stg[:, 7])
                nc.vector.tensor_add(acc[:], acc[:], h1[:])
                wsb = wconst.tile([P, O_LOC], BF16, name=f"wsb{t}")
                nc.vector.tensor_copy(wsb[:], acc[:])
                ws_bf.append(wsb)

            # ---- stream batch chunks: cast + silu + fused K=2048 matmul ----
            for ch in range(N_CHUNKS):
                xf = xstage.tile([P, KT, M_CHUNK], F32, name="xf", tag="xf")
                nc.sync.dma_start(xf[:], xt[ch])
                xb = xcat.tile([P, KT, M_CHUNK], BF16, name="xb", tag="xb")
                nc.vector.tensor_copy(xb[:], xf[:])
                sb = xcat.tile([P, KT, M_CHUNK], BF16, name="sb", tag="sb")
                nc.scalar.activation(sb[:], xf[:], AF.Silu)

                ot = opool.tile([P, J_SUB, O_LOC], BF16, name="ot")
                for j in range(J_SUB):
                    pt = psum.tile([P, O_LOC], F32, name="pt")
                    js = slice(P * j, P * (j + 1))
                    for k in range(KT):
                        nc.tensor.matmul(
                            pt[:], xb[:, k, js], wb_bf[k][:],
                            start=(k == 0), stop=False,
                        )
                    for k in range(KT):
                        nc.tensor.matmul(
                            pt[:], sb[:, k, js], ws_bf[k][:],
                            start=False, stop=(k == KT - 1),
                        )
                    nc.any.tensor_copy(ot[:, j], pt[:])
                nc.sync.dma_start(out[ch], ot[:])
    nc.compile()
    return nc


def _get_compiled():
    global _compiled
    if _compiled is None:
        _compiled = _build_kernel()
    return _compiled


def _shard_inputs(x, base_weight, spline_weight):
    """Full inputs -> 8 per-core in_maps (pure layout transforms)."""
    x = np.ascontiguousarray(np.asarray(x, dtype=np.float32))
    base_weight = np.ascontiguousarray(np.asarray(base_weight, dtype=np.float32))
    spline_weight = np.ascontiguousarray(np.asarray(spline_weight, dtype=np.float32))

    xt_full = np.ascontiguousarray(x.T)                     # [1024, 8192]
    btf = np.ascontiguousarray(base_weight.T)               # [1024, 1024]
    in_maps = []
    for core in range(N_CORES):
        r, c = divmod(core, C_SPLIT)
        osl = slice(O_LOC * c, O_LOC * (c + 1))
        xs = xt_full[:, B_LOC * r:B_LOC * (r + 1)]          # [1024, 4096]
        # [ch, p, it, b]: one contiguous 2MB block per chunk, 16KB rows
        xs6 = (xs.reshape(KT, P, N_CHUNKS, M_CHUNK)
                 .transpose(2, 1, 0, 3))
        btc = btf[:, osl].reshape(KT, P, O_LOC).transpose(1, 0, 2)
        stc = (spline_weight[osl]                      # [256 o, 1024 i, 8 g]
               .transpose(1, 2, 0)                     # [1024 i, 8 g, 256 o]
               .reshape(KT, P, G, O_LOC))
        in_maps.append({
            "xt": np.ascontiguousarray(xs6),
            "bt": np.ascontiguousarray(btc),
            "st": np.ascontiguousarray(stc),
        })
    return in_maps


def _gather_output(results):
    out = np.empty((8192, 1024), dtype=np.float32)
    for core in range(N_CORES):
        r, c = divmod(core, C_SPLIT)
        oc = results[core]["out"].astype(np.float32)   # [8 ch, 128 p, 4 j, 256 o]
        oc = oc.transpose(0, 2, 1, 3).reshape(B_LOC, O_LOC)
        out[B_LOC * r:B_LOC * (r + 1), O_LOC * c:O_LOC * (c + 1)] = oc
    return out


def run(trace=False, **inputs):
    """Run on the 8 NeuronCores; returns (out, BassKernelResults)."""
    nc = _get_compiled()
    in_maps = _shard_inputs(**inputs)
    res = run_bass_kernel_spmd(
        nc, in_maps, core_ids=list(range(N_CORES)), trace=trace)
    return _gather_output(res.results), res


def kernel(**inputs) -> np.ndarray:
    out, _ = run(trace=False, **inputs)
    return out



# revision 8
# speedup vs baseline: 1.1639x; 1.1639x over previous
"""KAN layer on 8 Trainium2 NeuronCores (Bass/Tile).

Computes out = x @ base_weight.T + silu(x) @ spline_weight.sum(-1).T
for x:[8192,1024] f32, base_weight:[1024,1024] f32,
spline_weight:[1024,1024,8] f32 -> out:[8192,1024] f32.

Strategy (self-contained, hardcoded for these shapes):
  * 2D shard over the 8 cores: batch split R=2, out-features split C=4.
    Core (r, c) computes out[4096r:4096(r+1), 256c:256(c+1)].
  * Host prep is layout + dtype narrowing only (transpose/reshape/slice
    plus f32->bf16 / f32->fp8e4m3 rounding -- the same rounding the
    device DVE would apply; all KAN math runs on device).  Narrow
    transfers cut per-core HBM traffic 28.3MB -> ~14.9MB, which was the
    baseline bottleneck (DMA busy ~79us at the ~358GB/s/core limit).
  * On-device per core: silu on ScalarE, spline g-axis reduce as DVE
    tree adds, x pair-cast bf16->fp8 on DVE.  Matmuls run
    weight-stationary with the batch stream as the moving operand
    (N=512 columns/matmul): the spline term (dominant magnitude) in
    bf16, the base term in fp8e4m3 DoubleRow (2x PE rate; base is only
    ~17% of output magnitude so the fp8 rounding contributes <1%
    relative error).  Both accumulate f32 into the same PSUM banks
    (K=2048 fused), 8 full banks, two batch passes of 2048.
  * Output is written bf16 and upcast to f32 on gather.
"""
import sys

for _p in ("/opt/trn_rl_repo",):
    if _p not in sys.path:
        sys.path.insert(0, _p)

import ml_dtypes
import numpy as np

import concourse.bass as bass  # noqa: F401  (bass must import before mybir use)
import concourse.mybir as mybir
import concourse.tile as tile
from concourse import bacc
from concourse.bass_utils import run_bass_kernel_spmd

P = 128
IN_F = 1024
G = 8
N_CORES = 8
R_SPLIT = 2
C_SPLIT = 4
B_LOC = 8192 // R_SPLIT      # 4096 batch rows per core
O_LOC = 1024 // C_SPLIT      # 256 out features per core
KT = IN_F // P               # 8 k-tiles over in_features
KS = KT // 2                 # 4 fp8 DoubleRow super-tiles (K=256 each)
N_MM = 512                   # moving (batch) columns per matmul = 1 PSUM bank
N_PASS = 2                   # batch passes of 2048
B_PASS = B_LOC // N_PASS     # 2048
J_SUB = B_PASS // N_MM       # 4 banks per o-tile per pass
O_TILES = O_LOC // P         # 2

F32 = mybir.dt.float32
BF16 = mybir.dt.bfloat16
F8E4 = mybir.dt.float8e4
AF = mybir.ActivationFunctionType
DR = mybir.MatmulPerfMode.DoubleRow

_compiled = None


def _build_kernel():
    nc = bacc.Bacc(None, target_bir_lowering=False, num_devices=N_CORES)
    xt = nc.dram_tensor("xt", [KT, P, B_LOC], BF16, kind="ExternalInput")
    st = nc.dram_tensor("st", [KS, P, 2, G, O_LOC], BF16, kind="ExternalInput")
    bt = nc.dram_tensor("bt", [P, KS, 2, O_LOC], F8E4, kind="ExternalInput")
    out = nc.dram_tensor("out", [O_TILES, P, B_LOC], BF16,
                         kind="ExternalOutput")

    with tile.TileContext(nc) as tc:
        with (
            tc.tile_pool(name="wconst", bufs=1) as wconst,
            tc.tile_pool(name="wstage", bufs=2) as wstage,
            tc.tile_pool(name="xstage", bufs=3) as xstage,
            tc.tile_pool(name="xres", bufs=1) as xres,
            tc.tile_pool(name="psum", bufs=1, space="PSUM") as psum,
            tc.tile_pool(name="opool", bufs=2) as opool,
        ):
            # ---- base weights: already fp8 DoubleRow layout, just land ----
            wb = wconst.tile([P, KS, 2, O_LOC], F8E4, name="wb")
            nc.sync.dma_start(wb[:], bt[:])

            # ---- spline weights: g-sum tree on DVE -> bf16 k-tile pairs ----
            ws = []
            for c in range(KS):
                stg = wstage.tile([P, 2, G, O_LOC], BF16, name="stg", tag="stg")
                nc.sync.dma_start(stg[:], st[c])
                t1 = wstage.tile([P, 2, 4, O_LOC], BF16, name="gs1", tag="gs1")
                nc.vector.tensor_add(t1[:], stg[:, :, 0:4], stg[:, :, 4:8])
                t2 = wstage.tile([P, 2, 2, O_LOC], BF16, name="gs2", tag="gs2")
                nc.vector.tensor_add(t2[:], t1[:, :, 0:2], t1[:, :, 2:4])
                wsc = wconst.tile([P, 2, O_LOC], BF16, name=f"ws{c}")
                nc.vector.tensor_add(wsc[:], t2[:, :, 0], t2[:, :, 1])
                ws.append(wsc)

            # ---- stream x k-tiles once: silu (resident) + fp8 pairs ----
            silu = []
            xq = [xres.tile([P, 2, B_LOC], F8E4, name=f"xq{s}")
                  for s in range(KS)]
            for k in range(KT):
                xf = xstage.tile([P, B_LOC], BF16, name="xf", tag="xf")
                nc.sync.dma_start(xf[:], xt[k])
                sl = xres.tile([P, B_LOC], BF16, name=f"sl{k}")
                nc.scalar.activation(sl[:], xf[:], AF.Silu)
                silu.append(sl)
                nc.vector.tensor_copy(xq[k // 2][:, k % 2], xf[:])

            # ---- weight-stationary matmuls: 2 passes x 8 banks ----
            for p in range(N_PASS):
                accs = [psum.tile([P, N_MM], F32, name=f"acc{o}_{j}")
                        for o in range(O_TILES) for j in range(J_SUB)]
                for k in range(KT):
                    for o in range(O_TILES):
                        osl = slice(P * o, P * (o + 1))
                        for j in range(J_SUB):
                            bsl = slice(B_PASS * p + N_MM * j,
                                        B_PASS * p + N_MM * (j + 1))
                            nc.tensor.matmul(
                                accs[J_SUB * o + j][:],
                                ws[k // 2][:, k % 2, osl],
                                silu[k][:, bsl],
                                start=(k == 0), stop=False,
                            )
                    if k % 2 == 1:
                        s = k // 2
                        for o in range(O_TILES):
                            osl = slice(P * o, P * (o + 1))
                            for j in range(J_SUB):
                                bsl = slice(B_PASS * p + N_MM * j,
                                            B_PASS * p + N_MM * (j + 1))
                                nc.tensor.matmul(
                                    accs[J_SUB * o + j][:],
                                    wb[:, s, :, osl],
                                    xq[s][:, :, bsl],
                                    start=False, stop=(s == KS - 1),
                                    perf_mode=DR,
                                )
                for o in range(O_TILES):
                    ot = opool.tile([P, J_SUB, N_MM], BF16, name="ot",
                                    tag=f"ot{o}")
                    for j in range(J_SUB):
                        nc.any.tensor_copy(ot[:, j], accs[J_SUB * o + j][:])
                    nc.sync.dma_start(
                        out[o, :, B_PASS * p:B_PASS * (p + 1)], ot[:])
    nc.compile()
    return nc


def _get_compiled():
    global _compiled
    if _compiled is None:
        _compiled = _build_kernel()
    return _compiled


def _shard_inputs(x, base_weight, spline_weight):
    """Full inputs -> 8 per-core in_maps (layout + dtype narrowing)."""
    x = np.ascontiguousarray(np.asarray(x, dtype=np.float32))
    base_weight = np.ascontiguousarray(np.asarray(base_weight, dtype=np.float32))
    spline_weight = np.ascontiguousarray(np.asarray(spline_weight, dtype=np.float32))

    # x.T -> [KT, P, 8192] bf16 (per-core batch slice below)
    xt_full = np.ascontiguousarray(x.T.reshape(KT, P, 8192)
                                   .astype(ml_dtypes.bfloat16))
    # base_weight.T [in, out] -> [P, KS, 2, out] fp8 (per-core out slice below)
    btf = (base_weight.T.reshape(KS, 2, P, 1024).transpose(2, 0, 1, 3)
           .astype(ml_dtypes.float8_e4m3))
    # spline [out, in, g] -> [in, g, out] -> [KS, P, 2, G, out] bf16
    stf = (spline_weight.transpose(1, 2, 0).reshape(KS, 2, P, G, 1024)
           .transpose(0, 2, 1, 3, 4).astype(ml_dtypes.bfloat16))

    in_maps = []
    for core in range(N_CORES):
        r, c = divmod(core, C_SPLIT)
        osl = slice(O_LOC * c, O_LOC * (c + 1))
        in_maps.append({
            "xt": np.ascontiguousarray(
                xt_full[:, :, B_LOC * r:B_LOC * (r + 1)]),
            "st": np.ascontiguousarray(stf[:, :, :, :, osl]),
            "bt": np.ascontiguousarray(btf[:, :, :, osl]),
        })
    return in_maps


def _gather_output(results):
    out = np.empty((8192, 1024), dtype=np.float32)
    for core in range(N_CORES):
        r, c = divmod(core, C_SPLIT)
        oc = results[core]["out"].astype(np.float32)   # [2 o, 128 p, 4096 b]
        oc = oc.reshape(O_LOC, B_LOC).T                # [4096 b, 256 o]
        out[B_LOC * r:B_LOC * (r + 1), O_LOC * c:O_LOC * (c + 1)] = oc
    return out


def run(trace=False, **inputs):
    """Run on the 8 NeuronCores; returns (out, BassKernelResults)."""
    nc = _get_compiled()
    in_maps = _shard_inputs(**inputs)
    res = run_bass_kernel_spmd(
        nc, in_maps, core_ids=list(range(N_CORES)), trace=trace)
    return _gather_output(res.results), res


def kernel(**inputs) -> np.ndarray:
    out, _ = run(trace=False, **inputs)
    return out


# revision 11
# speedup vs baseline: 1.4722x; 1.2649x over previous
"""KAN layer on 8 Trainium2 NeuronCores (Bass/Tile).

Computes out = x @ base_weight.T + silu(x) @ spline_weight.sum(-1).T
for x:[8192,1024] f32, base_weight:[1024,1024] f32,
spline_weight:[1024,1024,8] f32 -> out:[8192,1024] f32.

Strategy (self-contained, hardcoded for these shapes):
  * 2D shard over the 8 cores: batch split R=2, out-features split C=4.
    Core (r, c) computes out[4096r:4096(r+1), 256c:256(c+1)].
  * Host prep is layout + dtype narrowing only (transpose/reshape/slice
    plus f32->bf16 / f32->fp8e4m3 rounding -- the same rounding the
    device DVE would apply; all KAN math runs on device).  Narrow
    transfers cut per-core HBM traffic 28.3MB -> ~14.9MB, which was the
    baseline bottleneck (DMA busy ~79us at the ~358GB/s/core limit).
  * On-device per core: silu on ScalarE, spline g-axis reduce as DVE
    tree adds, x pair-cast bf16->fp8 on DVE.  Matmuls run
    weight-stationary with the batch stream as the moving operand
    (N=512 columns/matmul): the spline term (dominant magnitude) in
    bf16, the base term in fp8e4m3 DoubleRow (2x PE rate; base is only
    ~17% of output magnitude so the fp8 rounding contributes <1%
    relative error).  Both accumulate f32 into the same PSUM banks
    (K=2048 fused), 8 full banks, two batch passes of 2048.
  * Output is written bf16 and upcast to f32 on gather.
"""
import sys

for _p in ("/opt/trn_rl_repo",):
    if _p not in sys.path:
        sys.path.insert(0, _p)

import ml_dtypes
import numpy as np

import concourse.bass as bass  # noqa: F401  (bass must import before mybir use)
import concourse.mybir as mybir
import concourse.tile as tile
from concourse import bacc
from concourse.bass_utils import run_bass_kernel_spmd

P = 128
IN_F = 1024
G = 8
N_CORES = 8
R_SPLIT = 2
C_SPLIT = 4
B_LOC = 8192 // R_SPLIT      # 4096 batch rows per core
O_LOC = 1024 // C_SPLIT      # 256 out features per core
KT = IN_F // P               # 8 k-tiles over in_features
KS = KT // 2                 # 4 fp8 DoubleRow super-tiles (K=256 each)
N_MM = 512                   # moving (batch) columns per matmul = 1 PSUM bank
N_PASS = 2                   # batch passes of 2048
B_PASS = B_LOC // N_PASS     # 2048
J_SUB = B_PASS // N_MM       # 4 banks per o-tile per pass
O_TILES = O_LOC // P         # 2

F32 = mybir.dt.float32
BF16 = mybir.dt.bfloat16
F8E4 = mybir.dt.float8e4
AF = mybir.ActivationFunctionType
DR = mybir.MatmulPerfMode.DoubleRow

_compiled = None


def _build_kernel():
    nc = bacc.Bacc(None, target_bir_lowering=False, num_devices=N_CORES)
    xt = nc.dram_tensor("xt", [KT, N_PASS, P, B_PASS], BF16,
                        kind="ExternalInput")
    st = nc.dram_tensor("st", [KS, P, 2, G, O_LOC], BF16, kind="ExternalInput")
    bt = nc.dram_tensor("bt", [P, KS, 2, O_LOC], F8E4, kind="ExternalInput")
    out = nc.dram_tensor("out", [O_TILES, P, B_LOC], BF16,
                         kind="ExternalOutput")

    with tile.TileContext(nc) as tc:
        with (
            tc.tile_pool(name="wconst", bufs=1) as wconst,
            tc.tile_pool(name="wstage", bufs=2) as wstage,
            tc.tile_pool(name="xstage", bufs=3) as xstage,
            tc.tile_pool(name="xres", bufs=1) as xres,
            tc.tile_pool(name="psum", bufs=1, space="PSUM") as psum,
            tc.tile_pool(name="opool", bufs=2) as opool,
        ):
            wb = wconst.tile([P, KS, 2, O_LOC], F8E4, name="wb")
            ws = [None] * KS
            # silu / fp8-pair tiles, one per (k-tile, batch-half): separate
            # tiles so pass 0 never falsely depends on the half-1 stream.
            silu = [[None] * N_PASS for _ in range(KT)]
            xq = [[None] * N_PASS for _ in range(KS)]

            def stage_x(k, h):
                xf = xstage.tile([P, B_PASS], BF16, name="xf", tag="xf")
                nc.sync.dma_start(xf[:], xt[k, h])
                sl = xres.tile([P, B_PASS], BF16, name=f"sl{k}_{h}")
                nc.scalar.activation(sl[:], xf[:], AF.Silu)
                silu[k][h] = sl
                if xq[k // 2][h] is None:
                    xq[k // 2][h] = xres.tile([P, 2, B_PASS], F8E4,
                                              name=f"xq{k // 2}_{h}")
                nc.vector.tensor_copy(xq[k // 2][h][:, k % 2], xf[:])

            def stage_st(c):
                stg = wstage.tile([P, 2, G, O_LOC], BF16, name="stg",
                                  tag="stg")
                nc.sync.dma_start(stg[:], st[c])
                t1 = wstage.tile([P, 2, 4, O_LOC], BF16, name="gs1", tag="gs1")
                nc.vector.tensor_add(t1[:], stg[:, :, 0:4], stg[:, :, 4:8])
                t2 = wstage.tile([P, 2, 2, O_LOC], BF16, name="gs2", tag="gs2")
                nc.vector.tensor_add(t2[:], t1[:, :, 0:2], t1[:, :, 2:4])
                wsc = wconst.tile([P, 2, O_LOC], BF16, name=f"ws{c}")
                nc.vector.tensor_add(wsc[:], t2[:, :, 0], t2[:, :, 1])
                ws[c] = wsc

            # ---- pass-0 input stream, interleaved for earliest matmuls ----
            stage_x(0, 0)
            stage_st(0)
            nc.sync.dma_start(wb[:], bt[:])
            stage_x(1, 0)
            stage_st(1)
            stage_x(2, 0)
            stage_x(3, 0)
            stage_st(2)
            stage_x(4, 0)
            stage_x(5, 0)
            stage_st(3)
            stage_x(6, 0)
            stage_x(7, 0)

            def mm_pass(p):
                accs = [psum.tile([P, N_MM], F32, name=f"acc{o}_{j}")
                        for o in range(O_TILES) for j in range(J_SUB)]
                for k in range(KT):
                    for o in range(O_TILES):
                        osl = slice(P * o, P * (o + 1))
                        for j in range(J_SUB):
                            bsl = slice(N_MM * j, N_MM * (j + 1))
                            nc.tensor.matmul(
                                accs[J_SUB * o + j][:],
                                ws[k // 2][:, k % 2, osl],
                                silu[k][p][:, bsl],
                                start=(k == 0), stop=False,
                            )
                    if k % 2 == 1:
                        s = k // 2
                        for o in range(O_TILES):
                            osl = slice(P * o, P * (o + 1))
                            for j in range(J_SUB):
                                bsl = slice(N_MM * j, N_MM * (j + 1))
                                nc.tensor.matmul(
                                    accs[J_SUB * o + j][:],
                                    wb[:, s, :, osl],
                                    xq[s][p][:, :, bsl],
                                    start=False, stop=(s == KS - 1),
                                    perf_mode=DR,
                                )
                for o in range(O_TILES):
                    ot = opool.tile([P, J_SUB, N_MM], BF16, name="ot",
                                    tag=f"ot{o}")
                    for j in range(J_SUB):
                        nc.any.tensor_copy(ot[:, j], accs[J_SUB * o + j][:])
                    nc.sync.dma_start(
                        out[o, :, B_PASS * p:B_PASS * (p + 1)], ot[:])

            mm_pass(0)
            for k in range(KT):
                stage_x(k, 1)
            mm_pass(1)
    nc.compile()
    return nc


def _get_compiled():
    global _compiled
    if _compiled is None:
        _compiled = _build_kernel()
    return _compiled


def _shard_inputs(x, base_weight, spline_weight):
    """Full inputs -> 8 per-core in_maps (layout + dtype narrowing)."""
    x = np.ascontiguousarray(np.asarray(x, dtype=np.float32))
    base_weight = np.ascontiguousarray(np.asarray(base_weight, dtype=np.float32))
    spline_weight = np.ascontiguousarray(np.asarray(spline_weight, dtype=np.float32))

    # x.T -> [KT, P, 8192] bf16 (per-core batch slice + half split below)
    xt_full = np.ascontiguousarray(x.T.reshape(KT, P, 8192)
                                   .astype(ml_dtypes.bfloat16))
    # base_weight.T [in, out] -> [P, KS, 2, out] fp8 (per-core out slice below)
    btf = (base_weight.T.reshape(KS, 2, P, 1024).transpose(2, 0, 1, 3)
           .astype(ml_dtypes.float8_e4m3))
    # spline [out, in, g] -> [in, g, out] -> [KS, P, 2, G, out] bf16
    stf = (spline_weight.transpose(1, 2, 0).reshape(KS, 2, P, G, 1024)
           .transpose(0, 2, 1, 3, 4).astype(ml_dtypes.bfloat16))

    in_maps = []
    for core in range(N_CORES):
        r, c = divmod(core, C_SPLIT)
        osl = slice(O_LOC * c, O_LOC * (c + 1))
        xs = (xt_full[:, :, B_LOC * r:B_LOC * (r + 1)]
              .reshape(KT, P, N_PASS, B_PASS).transpose(0, 2, 1, 3))
        in_maps.append({
            "xt": np.ascontiguousarray(xs),
            "st": np.ascontiguousarray(stf[:, :, :, :, osl]),
            "bt": np.ascontiguousarray(btf[:, :, :, osl]),
        })
    return in_maps


def _gather_output(results):
    out = np.empty((8192, 1024), dtype=np.float32)
    for core in range(N_CORES):
        r, c = divmod(core, C_SPLIT)
        oc = results[core]["out"].astype(np.float32)   # [2 o, 128 p, 4096 b]
        oc = oc.reshape(O_LOC, B_LOC).T                # [4096 b, 256 o]
        out[B_LOC * r:B_LOC * (r + 1), O_LOC * c:O_LOC * (c + 1)] = oc
    return out


def run(trace=False, **inputs):
    """Run on the 8 NeuronCores; returns (out, BassKernelResults)."""
    nc = _get_compiled()
    in_maps = _shard_inputs(**inputs)
    res = run_bass_kernel_spmd(
        nc, in_maps, core_ids=list(range(N_CORES)), trace=trace)
    return _gather_output(res.results), res


def kernel(**inputs) -> np.ndarray:
    out, _ = run(trace=False, **inputs)
    return out
